# revision 7
# baseline (speedup 1.0000x reference)
"""Causal multi-head attention on 8 Trainium2 NeuronCores.

Problem (fp32): x [2,2048,1024]; Wq/Wk/Wv/Wo [1024,1024] (+biases);
16 heads x 64 dims; causal softmax attention.  ~68.7 GFLOP total.

Sharding: core c handles batch b = c//4 and head group g = c%4
(4 heads = 256 of the 1024 qkv dims).  Each core computes its partial
out = attn_heads(b, g) @ Wo[256 rows] in fp16 and the host sums the 4
partials per batch (fp32).  Biases: bq/bk applied on-device (they
affect softmax); bv on-device in the V projection; bo once on the host.

Device algorithm per core (transposed flash attention; scores are
~N(0,1) so fp32-accumulated exp needs no running-max subtraction).
All matmul operands are fp16 — 1 cyc/row on the PE at any N (measured
~2.37 rows/ns steady-state vs 2.4 peak), fp32 accumulation in PSUM:
  - every input is HOST-PRE-ARRANGED into its exact SBUF layout so
    each DMA is a plain [128, L] descriptor with 1-8KB contiguous
    lines.  The DMA ring is packet-rate-bound (~57ns/packet/engine),
    so the old scattered 256-512B-line rearranges made the initial
    load DMA-bound; contiguous layouts + first-use-ordered triggers
    (wq, xt half, wq, xt half, wk, ...) start the PE at ~11us
  - Q^T/K^T = W.T @ x.T with W chunks stationary (d on partitions);
    Q^T lands per-head with the sibling head's 64 partitions zeroed
    so scores can contract the full 128 partitions of K^T
  - V = x @ Wv with x^T chunks stationary; a ones column per head
    makes the PV matmul also produce the softmax denominators
  - scores^T[k,q] = K^T_chunk.T @ Q^T; exp on ACT (1/sqrt(dk) fused
    into the activation scale); causal 0/1 mask multiplies on DVE.
    Diagonal chunks are trimmed to their live columns (m=0:512,
    m=1:384, m=2:256, m=3:128 — fp16 has no N>=256 rate cliff)
  - out^T[v,q] += [V|1]_chunk.T @ expS^T accumulated in PSUM; row 64
    of the accumulator is the denominator
  - normalization: 1/s = exp(-ln s) on ACT, then a partition
    broadcast through a DRAM bounce (SBUF->DRAM->SBUF with a 0-stride
    source AP) and one DVE multiply for blocks 0-2 (latency hidden
    under the next block); K=1 broadcast matmuls for the last block
  - final partial = at.T @ Wo chunks -> fp16 out [2048, 1024].  On
    the last block the PSUM->SBUF drains alternate between ACT and
    DVE and each 512-column half is DMA'd as soon as it lands
  - PSUM pools pqk/pv/ps/po/pf = 1/1/3/2/1 banks; es (exp output)
    5-deep; o_sb staging 4-deep.  This topology is a sharp local optimum: pairing chunks
    into 2-bank scores tiles, deepening ps to 4 by folding pf in,
    sharing pools across phases, or weaving deferred work into the
    attention streams all measured 2-20% WORSE (in-order engine
    queues: anything that couples phases through pool slots, or puts
    an ACT-gated instruction in the PE queue mid-attention, stalls)

fp16 end-to-end rel err (vs fp32 reference, max-abs scale): ~6e-4.
Measured ~163-166us on a quiet device (the shared TRN2 drifts between
p-states: identical NEFFs measure 163-197us depending on neighbor
load — compare variants only via interleaved A/B in one process).

The _split_sync_waits post-pass works around the installed walrus
accepting only one sync wait command per instruction.
"""

import numpy as np

B, S, D = 2, 2048, 1024
H, DK, DV = 16, 64, 64
D_OUT = 1024
N_CORES = 8
H_LOC = H // 4          # 4 heads per core
DLOC = H_LOC * DK       # 256 qkv dims per core
NBLK = S // 512         # 4 query blocks of 512 tokens
NKB = S // 128          # 16 key chunks of 128 tokens

DT_MM_NAME = "float16"

# q0(m): first live query column of diagonal chunk m (columns below it
# are fully causally masked)
_Q0 = {0: 0, 1: 128, 2: 256, 3: 384}

_CACHE = {}


def _build_nc():
    import bass_rust
    import concourse.bass as bass
    import concourse.mybir as mybir
    import concourse.tile as tile
    from concourse.tile import add_dep_helper

    FP = mybir.dt.float32
    DT = getattr(mybir.dt, DT_MM_NAME)

    def _split_sync_waits(nc_):
        """The installed walrus accepts only ONE sync wait command per
        instruction; Tile emits several (worst on the exit drain). Hoist
        extra waits onto nop instructions inserted just before, on the
        same engine queue — in-order queue execution keeps semantics."""
        n = 0
        for f in nc_.m.functions:
            for bb in f.blocks:
                out = []
                for inst in bb.instructions:
                    si = inst.sync_info
                    waits = list(si.on_wait) if si and si.on_wait else []
                    if len(waits) > 1:
                        for w in waits[:-1]:
                            n += 1
                            nop = mybir.InstNoOp(
                                name=f"{inst.name}-wsplit{n}",
                                sync_info=bass_rust.SyncInfo(
                                    on_wait=[w], on_update=[]
                                ),
                                bass_nofuse=True,
                                engine=inst.engine,
                            )
                            nc_.register_instruction(nop, overwrite=True)
                            out.append(nop)
                        inst.sync_info = bass_rust.SyncInfo(
                            on_wait=waits[-1:], on_update=list(si.on_update or [])
                        )
                    out.append(inst)
                bb.instructions[:] = out

    nc = bass.Bass(target_bir_lowering=False)
    # fp32r tiles trip the low-precision accumulation guard; all matmul
    # accumulation is still fp32 in PSUM.
    nc._allow_low_precision_reason = "fp32r matmul inputs"

    xs_d = nc.dram_tensor("xs", [128, NBLK, 8, 512], DT, kind="ExternalInput")
    wq_d = nc.dram_tensor("wq", [128, 8, DLOC], DT, kind="ExternalInput")
    wk_d = nc.dram_tensor("wk", [128, 8, DLOC], DT, kind="ExternalInput")
    wv_d = nc.dram_tensor("wv", [128, 8, DLOC], DT, kind="ExternalInput")
    wo_d = nc.dram_tensor("wo", [128, 2, D_OUT], DT, kind="ExternalInput")
    bqk_d = nc.dram_tensor("bqk", [128, 4], FP, kind="ExternalInput")
    bvb_d = nc.dram_tensor("bvb", [128, DLOC], DT, kind="ExternalInput")
    msk_d = nc.dram_tensor("msk", [128, 4, 512], DT, kind="ExternalInput")
    out_d = nc.dram_tensor("out", [S, D_OUT], DT, kind="ExternalOutput")

    Exp = mybir.ActivationFunctionType.Exp
    Ln = mybir.ActivationFunctionType.Ln
    Copy = mybir.ActivationFunctionType.Copy

    with tile.TileContext(nc) as tc:
        from contextlib import ExitStack

        stack = ExitStack()
        with stack:
            cpool = stack.enter_context(tc.tile_pool(name="consts", bufs=1))
            ppool = stack.enter_context(tc.tile_pool(name="persist", bufs=1))
            xtpool = stack.enter_context(tc.tile_pool(name="xt", bufs=2))
            qtpool = stack.enter_context(tc.tile_pool(name="qt", bufs=2))
            atpool = stack.enter_context(tc.tile_pool(name="at", bufs=2))
            espool = stack.enter_context(tc.tile_pool(name="es", bufs=5))
            rpool = stack.enter_context(tc.tile_pool(name="rec", bufs=2))
            bcpool = stack.enter_context(tc.tile_pool(name="bcast", bufs=8))
            rdpool = stack.enter_context(tc.tile_pool(name="rdram", bufs=2, space="DRAM"))
            opool = stack.enter_context(tc.tile_pool(name="outs", bufs=4))
            # 8 PSUM banks: proj 2 (QKV proj + out-proj, [128,512] fp32
            # 1-bank slots) + ps 2x2 (score pairs [128,2,512]) + po 2
            projpool = stack.enter_context(tc.tile_pool(name="proj", bufs=2, space="PSUM"))
            pspool = stack.enter_context(tc.tile_pool(name="ps", bufs=2, space="PSUM"))
            popool = stack.enter_context(tc.tile_pool(name="po", bufs=2, space="PSUM"))
            # ---- constants ----
            wq_sb = cpool.tile([128, 8, DLOC], DT)
            wk_sb = cpool.tile([128, 8, DLOC], DT)
            wv_sb = cpool.tile([128, 8, DLOC], DT)
            wo_sb = cpool.tile([128, 2, D_OUT], DT)
            bqk_sb = cpool.tile([128, 4], FP)
            bvb_sb = cpool.tile([128, DLOC], DT)
            msk_sb = cpool.tile([128, 4, 512], DT)
            ones_fp = cpool.tile([128, 64], FP)
            nc.vector.memset(ones_fp[:], 1.0)
            ones_dt = cpool.tile([1, 64], DT)
            nc.vector.tensor_copy(ones_dt[:], ones_fp[0:1, :])
            xt0 = xtpool.tile([128, 8, 512], DT, name="xt0")
            nc.sync.dma_start(wq_sb[:, 0:4, :], wq_d[:, 0:4, :])
            nc.sync.dma_start(xt0[:, 0:4, :], xs_d[:, 0, 0:4, :])
            nc.sync.dma_start(wq_sb[:, 4:8, :], wq_d[:, 4:8, :])
            nc.sync.dma_start(xt0[:, 4:8, :], xs_d[:, 0, 4:8, :])
            nc.sync.dma_start(wk_sb[:, 0:4, :], wk_d[:, 0:4, :])
            nc.sync.dma_start(wk_sb[:, 4:8, :], wk_d[:, 4:8, :])
            nc.sync.dma_start(bqk_sb[:], bqk_d[:])
            nc.sync.dma_start(bvb_sb[:], bvb_d[:])
            nc.sync.dma_start(wv_sb[:, 0:4, :], wv_d[:, 0:4, :])
            nc.sync.dma_start(wv_sb[:, 4:8, :], wv_d[:, 4:8, :])
            nc.sync.dma_start(msk_sb[:], msk_d[:])

            # ---- persistent K^T / [V|1] ----
            kt_sb = [ppool.tile([128, S], DT, name=f"kt{i}") for i in range(2)]
            vsb = ppool.tile([128, NKB, 4 * 65], DT)
            # ones columns (written via DVE copy: memset can't encode f32r)
            for h in range(4):
                nc.vector.tensor_copy(
                    vsb[:, :, 65 * h + 64], ones_fp[:, 0:NKB]
                )

            xt_tiles = {0: xt0}
            for jb in range(NBLK):
                tok0 = jb * 512
                xt = xt_tiles[jb]
                # prefetch next block's x^T (slot freed two blocks back)
                if jb + 1 < NBLK:
                    xtn = xtpool.tile([128, 8, 512], DT, name=f"xt{jb + 1}")
                    nc.sync.dma_start(xtn[:, 0:4, :], xs_d[:, jb + 1, 0:4, :])
                    nc.sync.dma_start(xtn[:, 4:8, :], xs_d[:, jb + 1, 4:8, :])
                    xt_tiles[jb + 1] = xtn
                if jb == 0:
                    nc.sync.dma_start(wo_sb[:], wo_d[:])

                # ---- Q^T / K^T projections for this block ----
                # Q^T lands in head-pair tiles [128, 512] (head 2mt on
                # partitions 0-63, head 2mt+1 on 64-127) — scores are
                # row-tiled K=64 matmuls so no sibling zero-padding needed
                qt_t = [qtpool.tile([128, 512], DT, name=f"qt{i}") for i in range(2)]
                for wsb, bcol in ((wq_sb, 0), (wk_sb, 2)):
                    for mt in range(2):
                        pq = projpool.tile([128, 512], FP, name="pq", tag="pj")
                        for c in range(8):
                            nc.tensor.matmul(
                                pq[:],
                                wsb[:, c, 128 * mt : 128 * (mt + 1)],
                                xt[:, c, :],
                                start=(c == 0),
                                stop=(c == 7),
                            )
                        if bcol == 0:
                            nc.vector.tensor_scalar_add(
                                qt_t[mt][:],
                                pq[:],
                                bqk_sb[:, mt : mt + 1],
                            )
                        else:
                            nc.vector.tensor_scalar_add(
                                kt_sb[mt][:, tok0 : tok0 + 512],
                                pq[:],
                                bqk_sb[:, bcol + mt : bcol + mt + 1],
                            )

                # ---- V projection for this block's 4 key chunks ----
                for t in range(4):
                    kb = jb * 4 + t
                    pv = projpool.tile([128, DLOC], FP, name="pv", tag="pj")
                    for c in range(8):
                        nc.tensor.matmul(
                            pv[:],
                            xt[:, c, 128 * t : 128 * (t + 1)],
                            wv_sb[:, c, :],
                            start=(c == 0),
                            stop=(c == 7),
                        )
                    vdst = vsb[:, kb, :].rearrange("p (h w) -> p h w", h=4)[:, :, 0:64]
                    nc.vector.tensor_add(
                        vdst,
                        pv[:].rearrange("p (h w) -> p h w", h=4),
                        bvb_sb[:].rearrange("p (h w) -> p h w", h=4),
                    )

                # ---- attention for this query block ----
                # heads run in PAIRS: scores are TWO row-tiled K=64
                # matmuls (tile_position (0,0) and (64,0) — disjoint row
                # groups run CONCURRENTLY on the PE, halving score time
                # vs the old zero-padded K=128 contraction); exp is ONE
                # merged ACT pass over the 2-bank pair tile
                at = [atpool.tile([128, 512], DT, name=f"at{i}") for i in range(2)]
                nkc = 4 * (jb + 1)
                rec_t = rpool.tile([1, 4, 512], DT)
                for hp in range(2):
                    qt_h = qt_t[hp]
                    kt_h = kt_sb[hp]
                    po = [popool.tile([65, 512], FP, name=f"po{j}", tag="po") for j in range(2)]
                    for kc in range(nkc):
                        m = kc - 4 * jb
                        # diagonal chunks: columns below 128*m are fully
                        # masked -- skip them (fp16 streams 1 cyc/row at
                        # any N, so trim to the exact live range)
                        q0 = _Q0.get(m, 0)
                        ps = pspool.tile([128, 2, 512], FP)
                        for j in range(2):
                            nc.tensor.matmul(
                                ps[:, j, q0:512],
                                kt_h[64 * j : 64 * j + 64, 128 * kc : 128 * (kc + 1)],
                                qt_h[64 * j : 64 * j + 64, q0:512],
                                start=True,
                                stop=True,
                            )
                        es = espool.tile([128, 2, 512], DT)
                        nc.scalar.activation(
                            es[:, :, q0:512], ps[:, :, q0:512], Exp, scale=0.125
                        )
                        if m >= 0:
                            for j in range(2):
                                nc.vector.tensor_mul(
                                    es[:, j, q0:512],
                                    es[:, j, q0:512],
                                    msk_sb[:, m, q0:512],
                                )
                        for j in range(2):
                            h = 2 * hp + j
                            nc.tensor.matmul(
                                po[j][:, q0:512],
                                vsb[:, kc, 65 * h : 65 * (h + 1)],
                                es[:, j, q0:512],
                                start=(kc == 0),
                                stop=(kc == nkc - 1),
                            )
                    # stash unnormalized out^T + 1/sums; the normalizing
                    # broadcast matmuls run after ALL heads so the PE never
                    # stalls on a DVE reciprocal round-trip mid-attention
                    with tc.high_priority():
                        # 1/s as exp(-ln s) on ACT: the [1,512] DVE
                        # reciprocal costs 3.4us; two ACT passes cost 1.4us
                        # and share the exp table set
                        for j in range(2):
                            h = 2 * hp + j
                            lns = rpool.tile([1, 512], FP, name=f"lns{h}")
                            nc.scalar.activation(lns[:], po[j][64:65, :], Ln)
                            nc.scalar.activation(
                                rec_t[:, h, :], lns[:], Exp, scale=-1.0
                            )
                            nc.vector.tensor_copy(
                                at[hp][64 * j : 64 * j + 64, :], po[j][0:64, :]
                            )
                for h in range(4):
                    p0 = 64 * (h % 2)
                    at_h = at[h // 2][p0 : p0 + 64, :]
                    if jb < NBLK - 1:
                        # broadcast 1/s across partitions via a DRAM bounce:
                        # zero PE involvement, latency hidden under the next
                        # block's attention
                        rscr = rdpool.tile([1, 512], DT, name=f"rscr{h}")
                        rwr = nc.sync.dma_start(rscr[:], rec_t[:, h, :])
                        bc = bcpool.tile([128, 512], DT)
                        rrd = nc.sync.dma_start(
                            bc[p0 : p0 + 64, :],
                            rscr[:].partition_broadcast(64)[:, 0, :],
                        )
                        add_dep_helper(rrd.ins, rwr.ins, True, "rec DRAM bounce RAW")
                        nc.vector.tensor_mul(at_h, at_h, bc[p0 : p0 + 64, :])
                    else:
                        # last block: nothing hides the bounce latency and the
                        # PE is idle, so a K=1 broadcast matmul is faster
                        pbc = popool.tile([64, 512], FP, name="pbcl", tag="po")
                        nc.tensor.matmul(
                            pbc[:], ones_dt[:], rec_t[:, h, :], start=True, stop=True
                        )
                        nc.vector.tensor_mul(at_h, at_h, pbc[:])

                # ---- output projection for this block ----
                for qc in range(4):
                    o_sb = opool.tile([128, D_OUT], DT)
                    r0 = tok0 + 128 * qc
                    for dblk in range(2):
                        if jb < NBLK - 1:
                            pf = projpool.tile([128, 512], FP, name="pf", tag="pj")
                        else:
                            # attention is over: rotate through the idle
                            # scores banks so copies never stall the PE
                            pf = pspool.tile([128, 512], FP, name="pfl", tag="ps")
                        for vc in range(2):
                            nc.tensor.matmul(
                                pf[:],
                                at[vc][:, 128 * qc : 128 * (qc + 1)],
                                wo_sb[:, vc, 512 * dblk : 512 * (dblk + 1)],
                                start=(vc == 0),
                                stop=(vc == 1),
                            )
                        dsl = slice(512 * dblk, 512 * (dblk + 1))
                        if jb == NBLK - 1:
                            # last block: alternate the PSUM->SBUF drains
                            # between DVE and ACT and flush each half as
                            # soon as it lands, so the final DMA isn't
                            # serialized behind both copies
                            if dblk == 0:
                                nc.scalar.activation(
                                    o_sb[:, dsl], pf[:], Copy
                                )
                            else:
                                nc.vector.tensor_copy(o_sb[:, dsl], pf[:])
                            nc.sync.dma_start(
                                out_d[r0 : r0 + 128, dsl], o_sb[:, dsl]
                            )
                        else:
                            nc.vector.tensor_copy(o_sb[:, dsl], pf[:])
                    if jb < NBLK - 1:
                        nc.sync.dma_start(out_d[r0 : r0 + 128, :], o_sb[:])

    _split_sync_waits(nc)
    return nc


def _get_nc():
    if "nc" not in _CACHE:
        _CACHE["nc"] = _build_nc()
    return _CACHE["nc"]


def kernel(x, Wq, bq, Wk, bk, Wv, bv, Wo, bo, _trace=False):
    from concourse.bass_utils import run_bass_kernel_spmd

    if DT_MM_NAME == "bfloat16":
        import ml_dtypes

        np_dt = ml_dtypes.bfloat16
    elif DT_MM_NAME == "float16":
        np_dt = np.float16
    else:
        np_dt = np.float32

    x = np.asarray(x, dtype=np.float32)
    Wq, bq = np.asarray(Wq, np.float32), np.asarray(bq, np.float32)
    Wk, bk = np.asarray(Wk, np.float32), np.asarray(bk, np.float32)
    Wv, bv = np.asarray(Wv, np.float32), np.asarray(bv, np.float32)
    Wo, bo = np.asarray(Wo, np.float32), np.asarray(bo, np.float32)

    # causal 0/1 masks for the 4 diagonal positions of a 512-query block
    p = np.arange(128)[:, None, None]
    m = np.arange(4)[None, :, None]
    q = np.arange(512)[None, None, :]
    msk = (q >= p + 128 * m).astype(np.float32)

    def wlayout(W):  # [1024, 256] -> [128, 8, 256], chunk-major partitions
        return np.ascontiguousarray(
            W.reshape(8, 128, DLOC).transpose(1, 0, 2)
        ).astype(np_dt)

    in_maps = []
    for c in range(N_CORES):
        b, g = c // 4, c % 4
        s = slice(g * DLOC, (g + 1) * DLOC)
        bq_s, bk_s = bq[s], bk[s]
        bqk = np.stack(
            [bq_s[:128], bq_s[128:], bk_s[:128], bk_s[128:]], axis=1
        ).astype(np.float32)
        xs = np.ascontiguousarray(
            x[b].reshape(NBLK, 512, 8, 128).transpose(3, 0, 2, 1)
        ).astype(np_dt)
        wo_l = np.ascontiguousarray(
            Wo[s, :].reshape(2, 128, D_OUT).transpose(1, 0, 2)
        ).astype(np_dt)
        in_maps.append(
            {
                "xs": xs,
                "wq": wlayout(Wq[:, s]),
                "wk": wlayout(Wk[:, s]),
                "wv": wlayout(Wv[:, s]),
                "wo": wo_l,
                "bqk": bqk,
                "bvb": np.tile(bv[s][None, :], (128, 1)).astype(np_dt),
                "msk": msk.astype(np_dt),
            }
        )

    nc = _get_nc()
    res = run_bass_kernel_spmd(nc, in_maps, list(range(N_CORES)), trace=_trace)

    out = np.empty((B, S, D_OUT), dtype=np.float32)
    for b in range(B):
        acc = res.results[4 * b]["out"].astype(np.float32)
        for g in range(1, 4):
            acc = acc + res.results[4 * b + g]["out"].astype(np.float32)
        out[b] = acc + bo[None, :]
    if _trace:
        return out, res
    return out



# revision 8
# speedup vs baseline: 1.2464x; 1.2464x over previous
"""Causal multi-head attention on 8 Trainium2 NeuronCores.

Problem (fp32): x [2,2048,1024]; Wq/Wk/Wv/Wo [1024,1024] (+biases);
16 heads x 64 dims; causal softmax attention.  ~68.7 GFLOP total.

Sharding: core c handles batch b = c//4 and head group g = c%4
(4 heads = 256 of the 1024 qkv dims).  Each core computes its partial
out = attn_heads(b, g) @ Wo[256 rows] in fp16 and the host sums the 4
partials per batch (fp32).  Biases: bq/bk applied on-device (they
affect softmax); bv on-device in the V projection; bo once on the host.

Device algorithm per core (transposed flash attention; scores are
~N(0,1) so fp32-accumulated exp needs no running-max subtraction).
All matmul operands are fp16 — 1 cyc/row on the PE at any N (measured
~2.37 rows/ns steady-state vs 2.4 peak), fp32 accumulation in PSUM:
  - every input is HOST-PRE-ARRANGED into its exact SBUF layout so
    each DMA is a plain [128, L] descriptor with 1-8KB contiguous
    lines.  The DMA ring is packet-rate-bound (~57ns/packet/engine),
    so the old scattered 256-512B-line rearranges made the initial
    load DMA-bound; contiguous layouts + first-use-ordered triggers
    (wq, xt half, wq, xt half, wk, ...) start the PE at ~11us
  - Q^T/K^T = W.T @ x.T with W chunks stationary (d on partitions);
    Q^T lands per-head with the sibling head's 64 partitions zeroed
    so scores can contract the full 128 partitions of K^T
  - V = x @ Wv with x^T chunks stationary; a ones column per head
    makes the PV matmul also produce the softmax denominators
  - scores^T[k,q] = K^T_chunk.T @ Q^T; exp on ACT (1/sqrt(dk) fused
    into the activation scale); causal 0/1 mask multiplies on DVE.
    Diagonal chunks are trimmed to their live columns (m=0:512,
    m=1:384, m=2:256, m=3:128 — fp16 has no N>=256 rate cliff)
  - out^T[v,q] += [V|1]_chunk.T @ expS^T accumulated in PSUM; row 64
    of the accumulator is the denominator
  - normalization: 1/s = exp(-ln s) on ACT, then a partition
    broadcast through a DRAM bounce (SBUF->DRAM->SBUF with a 0-stride
    source AP) and one DVE multiply for blocks 0-2 (latency hidden
    under the next block); K=1 broadcast matmuls for the last block
  - final partial = at.T @ Wo chunks -> fp16 out [2048, 1024].  On
    the last block the PSUM->SBUF drains alternate between ACT and
    DVE and each 512-column half is DMA'd as soon as it lands
  - PSUM pools pqk/pv/ps/po/pf = 1/1/3/2/1 banks; es (exp output)
    5-deep; o_sb staging 4-deep.  This topology is a sharp local optimum: pairing chunks
    into 2-bank scores tiles, deepening ps to 4 by folding pf in,
    sharing pools across phases, or weaving deferred work into the
    attention streams all measured 2-20% WORSE (in-order engine
    queues: anything that couples phases through pool slots, or puts
    an ACT-gated instruction in the PE queue mid-attention, stalls)

fp16 end-to-end rel err (vs fp32 reference, max-abs scale): ~6e-4.
Measured ~163-166us on a quiet device (the shared TRN2 drifts between
p-states: identical NEFFs measure 163-197us depending on neighbor
load — compare variants only via interleaved A/B in one process).

The _split_sync_waits post-pass works around the installed walrus
accepting only one sync wait command per instruction.
"""

import numpy as np

B, S, D = 2, 2048, 1024
H, DK, DV = 16, 64, 64
D_OUT = 1024
N_CORES = 8
H_LOC = H // 4          # 4 heads per core
DLOC = H_LOC * DK       # 256 qkv dims per core
NBLK = S // 512         # 4 query blocks of 512 tokens
NKB = S // 128          # 16 key chunks of 128 tokens

DT_MM_NAME = "float16"

# q0(m): first live query column of diagonal chunk m (columns below it
# are fully causally masked)
_Q0 = {0: 0, 1: 128, 2: 256, 3: 384}

_CACHE = {}


def _build_nc():
    import bass_rust
    import concourse.bass as bass
    import concourse.mybir as mybir
    import concourse.tile as tile
    from concourse.tile import add_dep_helper

    FP = mybir.dt.float32
    DT = getattr(mybir.dt, DT_MM_NAME)

    def _split_sync_waits(nc_):
        """The installed walrus accepts only ONE sync wait command per
        instruction; Tile emits several (worst on the exit drain). Hoist
        extra waits onto nop instructions inserted just before, on the
        same engine queue — in-order queue execution keeps semantics."""
        n = 0
        for f in nc_.m.functions:
            for bb in f.blocks:
                out = []
                for inst in bb.instructions:
                    si = inst.sync_info
                    waits = list(si.on_wait) if si and si.on_wait else []
                    if len(waits) > 1:
                        for w in waits[:-1]:
                            n += 1
                            nop = mybir.InstNoOp(
                                name=f"{inst.name}-wsplit{n}",
                                sync_info=bass_rust.SyncInfo(
                                    on_wait=[w], on_update=[]
                                ),
                                bass_nofuse=True,
                                engine=inst.engine,
                            )
                            nc_.register_instruction(nop, overwrite=True)
                            out.append(nop)
                        inst.sync_info = bass_rust.SyncInfo(
                            on_wait=waits[-1:], on_update=list(si.on_update or [])
                        )
                    out.append(inst)
                bb.instructions[:] = out

    nc = bass.Bass(target_bir_lowering=False)
    # fp32r tiles trip the low-precision accumulation guard; all matmul
    # accumulation is still fp32 in PSUM.
    nc._allow_low_precision_reason = "fp32r matmul inputs"

    xs_d = nc.dram_tensor("xs", [128, NBLK, 8, 512], DT, kind="ExternalInput")
    wq_d = nc.dram_tensor("wq", [128, 8, DLOC], DT, kind="ExternalInput")
    wk_d = nc.dram_tensor("wk", [128, 8, DLOC], DT, kind="ExternalInput")
    wv_d = nc.dram_tensor("wv", [128, 8, DLOC], DT, kind="ExternalInput")
    wo_d = nc.dram_tensor("wo", [128, 2, D_OUT], DT, kind="ExternalInput")
    bqk_d = nc.dram_tensor("bqk", [128, 4], FP, kind="ExternalInput")
    bvb_d = nc.dram_tensor("bvb", [128, DLOC], DT, kind="ExternalInput")
    msk_d = nc.dram_tensor("msk", [128, 4, 512], DT, kind="ExternalInput")
    out_d = nc.dram_tensor("out", [S, D_OUT], DT, kind="ExternalOutput")

    Exp = mybir.ActivationFunctionType.Exp
    Ln = mybir.ActivationFunctionType.Ln
    Copy = mybir.ActivationFunctionType.Copy

    with tile.TileContext(nc) as tc:
        from contextlib import ExitStack

        stack = ExitStack()
        with stack:
            cpool = stack.enter_context(tc.tile_pool(name="consts", bufs=1))
            ppool = stack.enter_context(tc.tile_pool(name="persist", bufs=1))
            xtpool = stack.enter_context(tc.tile_pool(name="xt", bufs=2))
            qtpool = stack.enter_context(tc.tile_pool(name="qt", bufs=2))
            atpool = stack.enter_context(tc.tile_pool(name="at", bufs=2))
            espool = stack.enter_context(tc.tile_pool(name="es", bufs=5))
            rpool = stack.enter_context(tc.tile_pool(name="rec", bufs=2))
            bcpool = stack.enter_context(tc.tile_pool(name="bcast", bufs=8))
            rdpool = stack.enter_context(tc.tile_pool(name="rdram", bufs=2, space="DRAM"))
            opool = stack.enter_context(tc.tile_pool(name="outs", bufs=4))
            # 8 PSUM banks: qkv-proj 1 + score pairs 2x2 + po 2 +
            # out-proj 1.  pf must NOT share a pool with the projections:
            # out-proj waits on the norm chain, and a shared pool would
            # stop the scheduler hoisting the next block's projections
            # over that wait (7us PE hole per block boundary, measured)
            qkvpool = stack.enter_context(tc.tile_pool(name="qkv", bufs=1, space="PSUM"))
            pspool = stack.enter_context(tc.tile_pool(name="ps", bufs=2, space="PSUM"))
            popool = stack.enter_context(tc.tile_pool(name="po", bufs=2, space="PSUM"))
            pfpool = stack.enter_context(tc.tile_pool(name="pf", bufs=1, space="PSUM"))
            # ---- constants ----
            wq_sb = cpool.tile([128, 8, DLOC], DT)
            wk_sb = cpool.tile([128, 8, DLOC], DT)
            wv_sb = cpool.tile([128, 8, DLOC], DT)
            wo_sb = cpool.tile([128, 2, D_OUT], DT)
            bqk_sb = cpool.tile([128, 4], FP)
            bvb_sb = cpool.tile([128, DLOC], DT)
            msk_sb = cpool.tile([128, 4, 512], DT)
            ones_fp = cpool.tile([128, 64], FP)
            nc.vector.memset(ones_fp[:], 1.0)
            ones_dt = cpool.tile([1, 64], DT)
            nc.vector.tensor_copy(ones_dt[:], ones_fp[0:1, :])
            xt0 = xtpool.tile([128, 8, 512], DT, name="xt0")
            nc.sync.dma_start(wq_sb[:, 0:4, :], wq_d[:, 0:4, :])
            nc.sync.dma_start(xt0[:, 0:4, :], xs_d[:, 0, 0:4, :])
            nc.sync.dma_start(wq_sb[:, 4:8, :], wq_d[:, 4:8, :])
            nc.sync.dma_start(xt0[:, 4:8, :], xs_d[:, 0, 4:8, :])
            nc.sync.dma_start(wk_sb[:, 0:4, :], wk_d[:, 0:4, :])
            nc.sync.dma_start(wk_sb[:, 4:8, :], wk_d[:, 4:8, :])
            nc.sync.dma_start(bqk_sb[:], bqk_d[:])
            nc.sync.dma_start(bvb_sb[:], bvb_d[:])
            nc.sync.dma_start(wv_sb[:, 0:4, :], wv_d[:, 0:4, :])
            nc.sync.dma_start(wv_sb[:, 4:8, :], wv_d[:, 4:8, :])
            nc.sync.dma_start(msk_sb[:], msk_d[:])

            # ---- persistent K^T / [V|1] ----
            kt_sb = [ppool.tile([128, S], DT, name=f"kt{i}") for i in range(2)]
            vsb = ppool.tile([128, NKB, 4 * 65], DT)
            # ones columns (written via DVE copy: memset can't encode f32r)
            for h in range(4):
                nc.vector.tensor_copy(
                    vsb[:, :, 65 * h + 64], ones_fp[:, 0:NKB]
                )

            xt_tiles = {0: xt0}
            for jb in range(NBLK):
                tok0 = jb * 512
                xt = xt_tiles[jb]
                # prefetch next block's x^T (slot freed two blocks back)
                if jb + 1 < NBLK:
                    xtn = xtpool.tile([128, 8, 512], DT, name=f"xt{jb + 1}")
                    nc.sync.dma_start(xtn[:, 0:4, :], xs_d[:, jb + 1, 0:4, :])
                    nc.sync.dma_start(xtn[:, 4:8, :], xs_d[:, jb + 1, 4:8, :])
                    xt_tiles[jb + 1] = xtn
                if jb == 0:
                    nc.sync.dma_start(wo_sb[:], wo_d[:])

                # ---- Q^T / K^T projections for this block ----
                # Q^T lands in head-pair tiles [128, 512] (head 2mt on
                # partitions 0-63, head 2mt+1 on 64-127) — scores are
                # row-tiled K=64 matmuls so no sibling zero-padding needed
                qt_t = [qtpool.tile([128, 512], DT, name=f"qt{i}") for i in range(2)]
                for wsb, bcol in ((wq_sb, 0), (wk_sb, 2)):
                    for mt in range(2):
                        pq = qkvpool.tile([128, 512], FP, name="pq", tag="pj")
                        for c in range(8):
                            nc.tensor.matmul(
                                pq[:],
                                wsb[:, c, 128 * mt : 128 * (mt + 1)],
                                xt[:, c, :],
                                start=(c == 0),
                                stop=(c == 7),
                            )
                        if bcol == 0:
                            nc.vector.tensor_scalar_add(
                                qt_t[mt][:],
                                pq[:],
                                bqk_sb[:, mt : mt + 1],
                            )
                        else:
                            nc.vector.tensor_scalar_add(
                                kt_sb[mt][:, tok0 : tok0 + 512],
                                pq[:],
                                bqk_sb[:, bcol + mt : bcol + mt + 1],
                            )

                # ---- V projection for this block's 4 key chunks ----
                for t in range(4):
                    kb = jb * 4 + t
                    pv = qkvpool.tile([128, DLOC], FP, name="pv", tag="pj")
                    for c in range(8):
                        nc.tensor.matmul(
                            pv[:],
                            xt[:, c, 128 * t : 128 * (t + 1)],
                            wv_sb[:, c, :],
                            start=(c == 0),
                            stop=(c == 7),
                        )
                    vdst = vsb[:, kb, :].rearrange("p (h w) -> p h w", h=4)[:, :, 0:64]
                    nc.vector.tensor_add(
                        vdst,
                        pv[:].rearrange("p (h w) -> p h w", h=4),
                        bvb_sb[:].rearrange("p (h w) -> p h w", h=4),
                    )

                # ---- attention for this query block ----
                # heads run in PAIRS: scores are TWO row-tiled K=64
                # matmuls (tile_position (0,0) and (64,0) — disjoint row
                # groups run CONCURRENTLY on the PE, halving score time
                # vs the old zero-padded K=128 contraction); exp is ONE
                # merged ACT pass over the 2-bank pair tile
                at = [atpool.tile([128, 512], DT, name=f"at{i}") for i in range(2)]
                nkc = 4 * (jb + 1)
                rec_t = rpool.tile([1, 4, 512], DT)
                for hp in range(2):
                    qt_h = qt_t[hp]
                    kt_h = kt_sb[hp]
                    po = [popool.tile([65, 512], FP, name=f"po{j}", tag="po") for j in range(2)]
                    for kc in range(nkc):
                        m = kc - 4 * jb
                        # diagonal chunks: columns below 128*m are fully
                        # masked -- skip them (fp16 streams 1 cyc/row at
                        # any N, so trim to the exact live range)
                        q0 = _Q0.get(m, 0)
                        ps = pspool.tile([128, 2, 512], FP)
                        for j in range(2):
                            nc.tensor.matmul(
                                ps[:, j, q0:512],
                                kt_h[64 * j : 64 * j + 64, 128 * kc : 128 * (kc + 1)],
                                qt_h[64 * j : 64 * j + 64, q0:512],
                                start=True,
                                stop=True,
                            )
                        es = espool.tile([128, 2, 512], DT)
                        nc.scalar.activation(
                            es[:, :, q0:512], ps[:, :, q0:512], Exp, scale=0.125
                        )
                        if m >= 0:
                            for j in range(2):
                                nc.vector.tensor_mul(
                                    es[:, j, q0:512],
                                    es[:, j, q0:512],
                                    msk_sb[:, m, q0:512],
                                )
                        for j in range(2):
                            h = 2 * hp + j
                            nc.tensor.matmul(
                                po[j][:, q0:512],
                                vsb[:, kc, 65 * h : 65 * (h + 1)],
                                es[:, j, q0:512],
                                start=(kc == 0),
                                stop=(kc == nkc - 1),
                            )
                    # stash unnormalized out^T + 1/sums; the normalizing
                    # broadcast matmuls run after ALL heads so the PE never
                    # stalls on a DVE reciprocal round-trip mid-attention
                    with tc.high_priority():
                        # 1/s as exp(-ln s) on ACT: the [1,512] DVE
                        # reciprocal costs 3.4us; two ACT passes cost 1.4us
                        # and share the exp table set
                        for j in range(2):
                            h = 2 * hp + j
                            lns = rpool.tile([1, 512], FP, name=f"lns{h}")
                            nc.scalar.activation(lns[:], po[j][64:65, :], Ln)
                            nc.scalar.activation(
                                rec_t[:, h, :], lns[:], Exp, scale=-1.0
                            )
                            nc.vector.tensor_copy(
                                at[hp][64 * j : 64 * j + 64, :], po[j][0:64, :]
                            )
                for h in range(4):
                    p0 = 64 * (h % 2)
                    at_h = at[h // 2][p0 : p0 + 64, :]
                    if jb < NBLK - 1:
                        # broadcast 1/s across partitions via a DRAM bounce:
                        # zero PE involvement, latency hidden under the next
                        # block's attention
                        rscr = rdpool.tile([1, 512], DT, name=f"rscr{h}")
                        rwr = nc.sync.dma_start(rscr[:], rec_t[:, h, :])
                        bc = bcpool.tile([128, 512], DT)
                        rrd = nc.sync.dma_start(
                            bc[p0 : p0 + 64, :],
                            rscr[:].partition_broadcast(64)[:, 0, :],
                        )
                        add_dep_helper(rrd.ins, rwr.ins, True, "rec DRAM bounce RAW")
                        nc.vector.tensor_mul(at_h, at_h, bc[p0 : p0 + 64, :])
                    else:
                        # last block: nothing hides the bounce latency and the
                        # PE is idle, so a K=1 broadcast matmul is faster
                        pbc = popool.tile([64, 512], FP, name="pbcl", tag="po")
                        nc.tensor.matmul(
                            pbc[:], ones_dt[:], rec_t[:, h, :], start=True, stop=True
                        )
                        nc.vector.tensor_mul(at_h, at_h, pbc[:])

                # ---- output projection for this block ----
                for qc in range(4):
                    o_sb = opool.tile([128, D_OUT], DT)
                    r0 = tok0 + 128 * qc
                    for dblk in range(2):
                        if jb < NBLK - 1:
                            pf = pfpool.tile([128, 512], FP, name="pf")
                        else:
                            # attention is over: rotate through the idle
                            # scores banks so copies never stall the PE
                            pf = pspool.tile([128, 512], FP, name="pfl", tag="ps")
                        for vc in range(2):
                            nc.tensor.matmul(
                                pf[:],
                                at[vc][:, 128 * qc : 128 * (qc + 1)],
                                wo_sb[:, vc, 512 * dblk : 512 * (dblk + 1)],
                                start=(vc == 0),
                                stop=(vc == 1),
                            )
                        dsl = slice(512 * dblk, 512 * (dblk + 1))
                        if jb == NBLK - 1:
                            # last block: alternate the PSUM->SBUF drains
                            # between DVE and ACT and flush each half as
                            # soon as it lands, so the final DMA isn't
                            # serialized behind both copies
                            if dblk == 0:
                                nc.scalar.activation(
                                    o_sb[:, dsl], pf[:], Copy
                                )
                            else:
                                nc.vector.tensor_copy(o_sb[:, dsl], pf[:])
                            nc.sync.dma_start(
                                out_d[r0 : r0 + 128, dsl], o_sb[:, dsl]
                            )
                        else:
                            nc.vector.tensor_copy(o_sb[:, dsl], pf[:])
                    if jb < NBLK - 1:
                        nc.sync.dma_start(out_d[r0 : r0 + 128, :], o_sb[:])

    _split_sync_waits(nc)
    return nc


def _get_nc():
    if "nc" not in _CACHE:
        _CACHE["nc"] = _build_nc()
    return _CACHE["nc"]


def kernel(x, Wq, bq, Wk, bk, Wv, bv, Wo, bo, _trace=False):
    from concourse.bass_utils import run_bass_kernel_spmd

    if DT_MM_NAME == "bfloat16":
        import ml_dtypes

        np_dt = ml_dtypes.bfloat16
    elif DT_MM_NAME == "float16":
        np_dt = np.float16
    else:
        np_dt = np.float32

    x = np.asarray(x, dtype=np.float32)
    Wq, bq = np.asarray(Wq, np.float32), np.asarray(bq, np.float32)
    Wk, bk = np.asarray(Wk, np.float32), np.asarray(bk, np.float32)
    Wv, bv = np.asarray(Wv, np.float32), np.asarray(bv, np.float32)
    Wo, bo = np.asarray(Wo, np.float32), np.asarray(bo, np.float32)

    # causal 0/1 masks for the 4 diagonal positions of a 512-query block
    p = np.arange(128)[:, None, None]
    m = np.arange(4)[None, :, None]
    q = np.arange(512)[None, None, :]
    msk = (q >= p + 128 * m).astype(np.float32)

    def wlayout(W):  # [1024, 256] -> [128, 8, 256], chunk-major partitions
        return np.ascontiguousarray(
            W.reshape(8, 128, DLOC).transpose(1, 0, 2)
        ).astype(np_dt)

    in_maps = []
    for c in range(N_CORES):
        b, g = c // 4, c % 4
        s = slice(g * DLOC, (g + 1) * DLOC)
        bq_s, bk_s = bq[s], bk[s]
        bqk = np.stack(
            [bq_s[:128], bq_s[128:], bk_s[:128], bk_s[128:]], axis=1
        ).astype(np.float32)
        xs = np.ascontiguousarray(
            x[b].reshape(NBLK, 512, 8, 128).transpose(3, 0, 2, 1)
        ).astype(np_dt)
        wo_l = np.ascontiguousarray(
            Wo[s, :].reshape(2, 128, D_OUT).transpose(1, 0, 2)
        ).astype(np_dt)
        in_maps.append(
            {
                "xs": xs,
                "wq": wlayout(Wq[:, s]),
                "wk": wlayout(Wk[:, s]),
                "wv": wlayout(Wv[:, s]),
                "wo": wo_l,
                "bqk": bqk,
                "bvb": np.tile(bv[s][None, :], (128, 1)).astype(np_dt),
                "msk": msk.astype(np_dt),
            }
        )

    nc = _get_nc()
    res = run_bass_kernel_spmd(nc, in_maps, list(range(N_CORES)), trace=_trace)

    out = np.empty((B, S, D_OUT), dtype=np.float32)
    for b in range(B):
        acc = res.results[4 * b]["out"].astype(np.float32)
        for g in range(1, 4):
            acc = acc + res.results[4 * b + g]["out"].astype(np.float32)
        out[b] = acc + bo[None, :]
    if _trace:
        return out, res
    return out

